# revision 4
# baseline (speedup 1.0000x reference)
"""MQA attention kernel for nn_Attention_37366215475332 on 8 Trainium2 cores.

Contract: kernel(**inputs) takes FULL unsharded inputs and returns the FULL
output. Sharding (hardcoded): batch data-parallel (2 groups) x tensor-parallel
over query heads (4-way, 8 heads/core); the single shared KV head is
replicated; w_qkv column-sharded on the query portion; w_dense row-sharded,
with the TP partial sums reduced on host.

Per-core device program (Bass/Tile, bf16 matmuls, fp32 PSUM accumulation):
  1. QKV projections from host-pre-transposed hidden (hidT), producing
     qT/kT in [head_dim, seq] layout plus RoPE applied on-chip, and v in
     natural [seq, head_dim] layout with a fused ones-column (for softmax
     denominators).
  2. Causal MQA attention computed transposed: scoresT[j,i] blocks via PE
     (two heads packed per 128-row array pass), exp on the Scalar engine
     straight out of PSUM (max-subtraction skipped; |scores*scale| < ~6),
     diagonal-block causal masking via precomputed binary masks, then
     ctxT/sums via PE with a [v | 1] stationary operand.
  3. Normalization by the softmax denominator (K=1 broadcast matmul + DVE
     multiply) and the row-sharded dense projection.

Hardcoded problem shapes: B=2, S=2048, HID=2048, NH=32, HD=64, rope base
10000.
"""

import math
import os
import sys

import numpy as np

B, S, HID = 2, 2048, 2048
NH, HD = 32, 64
ROPE_BASE = 10000
N_CORES = 8
DP = 2
TP = N_CORES // DP          # 4
HPC = NH // TP              # 8 heads per core
QCOLS = HPC * HD            # 512 query columns per core
NKT = HID // 128            # 16 contraction tiles
NSB = S // 512              # 4 seq blocks of 512
NST = S // 128              # 16 seq tiles of 128
SCALE = 1.0 / math.sqrt(HD)

_prog = None                # cached compiled Bass program
last_exec_time_ns = None    # set when BASS_KERNEL_TRACE=1


def _bf16(a):
    import ml_dtypes

    return np.asarray(a, dtype=ml_dtypes.bfloat16)


def _host_tables():
    """cos2/nsin2 [128, S] fp32 rope tables (pattern repeats every 64 rows),
    and the 4 diagonal-block causal masks [128, 512] (bf16 0/1)."""
    inv = 1.0 / (ROPE_BASE ** (np.arange(0, HD, 2, dtype=np.float32) / HD))  # [32]
    s = np.arange(S, dtype=np.float32)
    ang = s[None, :] * inv[:, None]          # [32, S]
    cos32 = np.cos(ang).astype(np.float32)
    sin32 = np.sin(ang).astype(np.float32)
    cos64 = np.concatenate([cos32, cos32], 0)            # [64, S]
    nsin64 = np.concatenate([-sin32, sin32], 0)          # [64, S]
    cos2 = np.concatenate([cos64, cos64], 0)             # [128, S]
    nsin2 = np.concatenate([nsin64, nsin64], 0)          # [128, S]

    jj = np.arange(128)[:, None]
    ii = np.arange(512)[None, :]
    masks = np.stack(
        [(128 * r + jj <= ii).astype(np.float32) for r in range(4)], 0
    )  # [4,128,512]
    return cos2, nsin2, _bf16(masks)


def _build_program():
    for p in ("/opt/trn_rl_repo", "/root/.axon_site/_ro/trn_rl_repo"):
        if os.path.isdir(p) and p not in sys.path:
            sys.path.append(p)
    import concourse.mybir as mybir
    import concourse.tile as tile
    from concourse import bacc

    F32 = mybir.dt.float32
    BF16 = mybir.dt.bfloat16
    EXP = mybir.ActivationFunctionType.Exp
    MUL = mybir.AluOpType.mult

    nc = bacc.Bacc("TRN2", target_bir_lowering=False, debug=False,
                   num_devices=N_CORES)

    hidT = nc.dram_tensor("hidT", [HID, S], BF16, kind="ExternalInput")
    w_q = nc.dram_tensor("w_q", [HID, QCOLS], BF16, kind="ExternalInput")
    w_kk = nc.dram_tensor("w_kk", [HID, 128], BF16, kind="ExternalInput")
    w_v = nc.dram_tensor("w_v", [HID, HD], BF16, kind="ExternalInput")
    w_d = nc.dram_tensor("w_d", [QCOLS, HID], BF16, kind="ExternalInput")
    cos2 = nc.dram_tensor("cos2", [128, S], F32, kind="ExternalInput")
    nsin2 = nc.dram_tensor("nsin2", [128, S], F32, kind="ExternalInput")
    mask4 = nc.dram_tensor("mask4", [4, 128, 512], BF16, kind="ExternalInput")
    out = nc.dram_tensor("out", [S, HID], BF16, kind="ExternalOutput")

    with tile.TileContext(nc) as tc:
        with (
            tc.tile_pool(name="const", bufs=1) as cpool,
            tc.tile_pool(name="hid", bufs=1) as hpool,
            tc.tile_pool(name="work", bufs=3) as wpool,
            tc.tile_pool(name="exps", bufs=6) as epool,
            tc.tile_pool(name="psA", bufs=2, space="PSUM") as psA,
            tc.tile_pool(name="psS", bufs=4, space="PSUM") as psS,
        ):
            # ---- resident constants -------------------------------------
            wq_sb = cpool.tile([128, NKT, QCOLS], BF16)
            nc.sync.dma_start(wq_sb[:], w_q.rearrange("(k p) c -> p k c", p=128))
            wkk_sb = cpool.tile([128, NKT, 128], BF16)
            nc.sync.dma_start(wkk_sb[:], w_kk.rearrange("(k p) c -> p k c", p=128))
            wv_sb = cpool.tile([128, NKT, HD], BF16)
            nc.sync.dma_start(wv_sb[:], w_v.rearrange("(k p) c -> p k c", p=128))
            wd_sb = cpool.tile([128, QCOLS // 128, HID], BF16)
            nc.sync.dma_start(wd_sb[:], w_d.rearrange("(k p) c -> p k c", p=128))
            cos_sb = cpool.tile([128, S], F32)
            nc.sync.dma_start(cos_sb[:], cos2[:])
            nsin_sb = cpool.tile([128, S], F32)
            nc.sync.dma_start(nsin_sb[:], nsin2[:])
            mask_sb = cpool.tile([128, 4, 512], BF16)
            nc.sync.dma_start(mask_sb[:], mask4.rearrange("r p c -> p r c"))
            ones_sb = cpool.tile([1, 64], F32)
            nc.vector.memset(ones_sb[:], 1.0)

            hid_sb = hpool.tile([128, NKT, S], BF16)
            nc.sync.dma_start(hid_sb[:], hidT.rearrange("(k p) s -> p k s", p=128))

            # persistent activations
            qT = [cpool.tile([128, S], BF16, tag=f"qT{m}", name=f"qT{m}")
                  for m in range(4)]
            kT = cpool.tile([128, S], BF16, tag="kT")
            v1 = cpool.tile([128, NST, HD + 1], BF16, tag="v1")
            ctx = [cpool.tile([128, S], BF16, tag=f"ctx{m}", name=f"ctx{m}")
                   for m in range(4)]

            def rope(dst, ps, nb):
                """dst[:, nb*512:+512] = rope(ps) with ps=[128,512] psum."""
                sl = slice(nb * 512, nb * 512 + 512)
                t1 = wpool.tile([128, 512], F32, tag="t1")
                nc.vector.tensor_tensor(t1[:], ps[:], cos_sb[:, sl], MUL)
                t2 = wpool.tile([128, 512], F32, tag="t2")
                for g in (0, 64):
                    nc.vector.tensor_tensor(
                        t2[g : g + 32], ps[g + 32 : g + 64], nsin_sb[g : g + 32, sl],
                        MUL)
                    nc.vector.tensor_tensor(
                        t2[g + 32 : g + 64], ps[g : g + 32],
                        nsin_sb[g + 32 : g + 64, sl], MUL)
                nc.vector.tensor_add(dst[:, sl], t1[:], t2[:])

            # ---- phase 1: projections + rope ----------------------------
            for nb in range(NSB):
                sl = slice(nb * 512, nb * 512 + 512)
                for mt in range(4):  # q
                    ps = psA.tile([128, 512], F32, tag="mm")
                    for kt in range(NKT):
                        nc.tensor.matmul(
                            ps[:], wq_sb[:, kt, mt * 128 : mt * 128 + 128],
                            hid_sb[:, kt, sl],
                            start=(kt == 0), stop=(kt == NKT - 1))
                    rope(qT[mt], ps, nb)
                ps = psA.tile([128, 512], F32, tag="mm")  # k duplicated
                for kt in range(NKT):
                    nc.tensor.matmul(
                        ps[:], wkk_sb[:, kt, :], hid_sb[:, kt, sl],
                        start=(kt == 0), stop=(kt == NKT - 1))
                rope(kT, ps, nb)

            for st in range(NST):  # v natural layout + ones column
                ps = psA.tile([128, 512], F32, tag="mm")
                for kt in range(NKT):
                    nc.tensor.matmul(
                        ps[:, :HD], hid_sb[:, kt, st * 128 : st * 128 + 128],
                        wv_sb[:, kt, :],
                        start=(kt == 0), stop=(kt == NKT - 1))
                nc.vector.tensor_copy(v1[:, st, :HD], ps[:, :HD])
                nc.vector.memset(v1[:, st, HD:], 1.0)

            # ---- phase 2: attention --------------------------------------
            for hp in range(4):          # head pairs (2 heads each)
                for ib in range(NSB):    # query blocks of 512
                    isl = slice(ib * 512, ib * 512 + 512)
                    n_jt = 4 * (ib + 1)
                    pc = [psA.tile([128, 512], F32, tag="ctx", name=f"pc{_i}")
                          for _i in range(2)]
                    for jt in range(n_jt):
                        jsl = slice(jt * 128, jt * 128 + 128)
                        for h2 in range(2):
                            g = slice(h2 * 64, h2 * 64 + 64)
                            pss = psS.tile([128, 512], F32, tag="sc")
                            nc.tensor.matmul(
                                pss[:], kT[g, jsl], qT[hp][g, isl],
                                start=True, stop=True,
                                tile_position=(h2 * 64, 0))
                            ex = epool.tile([128, 512], BF16, tag="ex")
                            nc.scalar.activation(ex[:], pss[:], EXP, scale=SCALE)
                            r = jt - 4 * ib
                            if r >= 0:
                                nc.vector.tensor_tensor(
                                    ex[:], ex[:], mask_sb[:, r, :], MUL)
                            nc.tensor.matmul(
                                pc[h2][: HD + 1], v1[:, jt, :], ex[:],
                                start=(jt == 0), stop=(jt == n_jt - 1))
                    for h2 in range(2):
                        rec = wpool.tile([1, 512], F32, tag="rec")
                        nc.vector.reciprocal(rec[:], pc[h2][HD : HD + 1, :])
                        pb = psA.tile([64, 512], F32, tag="mm")
                        nc.tensor.matmul(pb[:], ones_sb[:], rec[:],
                                         start=True, stop=True)
                        pbs = wpool.tile([64, 512], F32, tag="pbs")
                        nc.any.tensor_copy(pbs[:], pb[:])
                        nc.vector.tensor_tensor(
                            ctx[hp][h2 * 64 : h2 * 64 + 64, isl],
                            pc[h2][:HD], pbs[:], MUL)

            # ---- phase 3: dense ------------------------------------------
            for it in range(NST):
                tsl = slice(it * 128, it * 128 + 128)
                for nb in range(NSB):
                    ps = psA.tile([128, 512], F32, tag="mm")
                    for kt in range(4):
                        nc.tensor.matmul(
                            ps[:], ctx[kt][:, tsl],
                            wd_sb[:, kt, nb * 512 : nb * 512 + 512],
                            start=(kt == 0), stop=(kt == 3))
                    ob = wpool.tile([128, 512], BF16, tag="ob")
                    nc.any.tensor_copy(ob[:], ps[:])
                    nc.sync.dma_start(out[tsl, nb * 512 : nb * 512 + 512], ob[:])

    nc.compile()
    return nc


def _get_prog():
    global _prog
    if _prog is None:
        _prog = _build_program()
    return _prog


def kernel(hidden_states, w_qkv, w_dense):
    hidden_states = np.asarray(hidden_states, dtype=np.float32)
    w_qkv = np.asarray(w_qkv, dtype=np.float32)
    w_dense = np.asarray(w_dense, dtype=np.float32)

    nc = _get_prog()
    from concourse.bass_utils import run_bass_kernel_spmd

    cos2, nsin2, mask4 = _host_tables()
    w_k = w_qkv[:, NH * HD : NH * HD + HD]
    w_v = w_qkv[:, NH * HD + HD :]
    w_kk = _bf16(np.concatenate([w_k, w_k], axis=1))
    w_v = _bf16(w_v)

    in_maps = []
    for core in range(N_CORES):
        b, t = divmod(core, TP)
        c0 = t * QCOLS
        in_maps.append({
            "hidT": _bf16(hidden_states[b].T),
            "w_q": _bf16(w_qkv[:, c0 : c0 + QCOLS]),
            "w_kk": w_kk,
            "w_v": w_v,
            "w_d": _bf16(w_dense[c0 : c0 + QCOLS, :]),
            "cos2": cos2,
            "nsin2": nsin2,
            "mask4": mask4,
        })

    trace = os.environ.get("BASS_KERNEL_TRACE", "") == "1"
    kw = {}
    if trace:
        kw = {"trace": True, "tmpdir": os.environ.get("BASS_KERNEL_TRACE_DIR")
              or None}
    res = run_bass_kernel_spmd(nc, in_maps, list(range(N_CORES)), **kw)
    global last_exec_time_ns
    last_exec_time_ns = res.exec_time_ns

    outp = np.zeros((B, S, HID), dtype=np.float32)
    for core in range(N_CORES):
        b = core // TP
        outp[b] += np.asarray(res.results[core]["out"], dtype=np.float32)
    return outp


# revision 6
# speedup vs baseline: 1.0801x; 1.0801x over previous
"""MQA attention kernel for nn_Attention_37366215475332 on 8 Trainium2 cores.

Contract: kernel(**inputs) takes FULL unsharded inputs and returns the FULL
output. Sharding (hardcoded): batch data-parallel (2 groups) x tensor-parallel
over query heads (4-way, 8 heads/core); the single shared KV head is
replicated; w_qkv column-sharded on the query portion; w_dense row-sharded,
with the TP partial sums reduced on host.

Per-core device program (Bass/Tile, bf16 matmuls, fp32 PSUM accumulation):
  1. QKV projections from host-pre-transposed hidden (hidT), producing
     qT/kT in [head_dim, seq] layout with RoPE applied on-chip, and v in
     natural [seq, head_dim] layout with a fused ones-column (softmax
     denominators ride the context matmul for free).
  2. Causal MQA attention computed transposed: scoresT[j,i] blocks on the
     PE (two heads packed per 128-row pass via tile_position), exp on the
     Scalar engine straight out of PSUM (max-subtraction skipped;
     |scores*scale| < ~6), causal masking only on the 128x128 triangular
     chunk of each diagonal block, ctxT/sums accumulated on the PE with a
     [v | 1] stationary operand over the causally valid column range only.
  3. Normalization by the softmax denominator (K=1 broadcast matmul + DVE
     multiply) and the row-sharded dense projection.

Emission order interleaves per-head-group attention with the remaining q
projections so the PE always has independent work while ACT runs exps.

Hardcoded problem shapes: B=2, S=2048, HID=2048, NH=32, HD=64, rope base
10000.
"""

import math
import os
import sys

import numpy as np

B, S, HID = 2, 2048, 2048
NH, HD = 32, 64
ROPE_BASE = 10000
N_CORES = 8
DP = 2
TP = N_CORES // DP          # 4
HPC = NH // TP              # 8 heads per core
QCOLS = HPC * HD            # 512 query columns per core
NKT = HID // 128            # 16 contraction tiles
NSB = S // 512              # 4 seq blocks of 512
NST = S // 128              # 16 seq tiles of 128
SCALE = 1.0 / math.sqrt(HD)

_prog = None                # cached compiled Bass program
last_exec_time_ns = None    # set when BASS_KERNEL_TRACE=1


def _bf16(a):
    import ml_dtypes

    return np.asarray(a, dtype=ml_dtypes.bfloat16)


def _host_tables():
    """cos2/nsin2 [128, S] fp32 rope tables (pattern repeats every 64 rows),
    plus the [128, 128] triangular causal mask (bf16 0/1, keep j<=i)."""
    inv = 1.0 / (ROPE_BASE ** (np.arange(0, HD, 2, dtype=np.float32) / HD))  # [32]
    s = np.arange(S, dtype=np.float32)
    ang = s[None, :] * inv[:, None]          # [32, S]
    cos32 = np.cos(ang).astype(np.float32)
    sin32 = np.sin(ang).astype(np.float32)
    cos64 = np.concatenate([cos32, cos32], 0)            # [64, S]
    nsin64 = np.concatenate([-sin32, sin32], 0)          # [64, S]
    cos2 = np.concatenate([cos64, cos64], 0)             # [128, S]
    nsin2 = np.concatenate([nsin64, nsin64], 0)          # [128, S]

    jj = np.arange(128)[:, None]
    ii = np.arange(128)[None, :]
    tri = (jj <= ii).astype(np.float32)                  # [128, 128]
    return cos2, nsin2, _bf16(tri)


def _build_program():
    for p in ("/opt/trn_rl_repo", "/root/.axon_site/_ro/trn_rl_repo"):
        if os.path.isdir(p) and p not in sys.path:
            sys.path.append(p)
    import concourse.mybir as mybir
    import concourse.tile as tile
    from concourse import bacc

    F32 = mybir.dt.float32
    BF16 = mybir.dt.bfloat16
    EXP = mybir.ActivationFunctionType.Exp
    MUL = mybir.AluOpType.mult

    nc = bacc.Bacc("TRN2", target_bir_lowering=False, debug=False,
                   num_devices=N_CORES)

    hidT = nc.dram_tensor("hidT", [HID, S], BF16, kind="ExternalInput")
    w_q = nc.dram_tensor("w_q", [HID, QCOLS], BF16, kind="ExternalInput")
    w_kk = nc.dram_tensor("w_kk", [HID, 128], BF16, kind="ExternalInput")
    w_v = nc.dram_tensor("w_v", [HID, HD], BF16, kind="ExternalInput")
    w_d = nc.dram_tensor("w_d", [QCOLS, HID], BF16, kind="ExternalInput")
    cos2 = nc.dram_tensor("cos2", [128, S], F32, kind="ExternalInput")
    nsin2 = nc.dram_tensor("nsin2", [128, S], F32, kind="ExternalInput")
    trimask = nc.dram_tensor("trimask", [128, 128], BF16, kind="ExternalInput")
    out = nc.dram_tensor("out", [S, HID], BF16, kind="ExternalOutput")

    with tile.TileContext(nc) as tc:
        with (
            tc.tile_pool(name="const", bufs=1) as cpool,
            tc.tile_pool(name="hid", bufs=1) as hpool,
            tc.tile_pool(name="work", bufs=3) as wpool,
            tc.tile_pool(name="exps", bufs=8) as epool,
            tc.tile_pool(name="psM", bufs=2, space="PSUM") as psM,
            tc.tile_pool(name="psC", bufs=4, space="PSUM") as psC,
            tc.tile_pool(name="psS", bufs=2, space="PSUM") as psS,
        ):
            # ---- resident constants -------------------------------------
            wq_sb = cpool.tile([128, NKT, QCOLS], BF16)
            nc.sync.dma_start(wq_sb[:], w_q.rearrange("(k p) c -> p k c", p=128))
            wkk_sb = cpool.tile([128, NKT, 128], BF16)
            nc.sync.dma_start(wkk_sb[:], w_kk.rearrange("(k p) c -> p k c", p=128))
            wv_sb = cpool.tile([128, NKT, HD], BF16)
            nc.sync.dma_start(wv_sb[:], w_v.rearrange("(k p) c -> p k c", p=128))
            wd_sb = cpool.tile([128, QCOLS // 128, HID], BF16)
            nc.sync.dma_start(wd_sb[:], w_d.rearrange("(k p) c -> p k c", p=128))
            cos_sb = cpool.tile([128, S], F32)
            nc.sync.dma_start(cos_sb[:], cos2[:])
            nsin_sb = cpool.tile([128, S], F32)
            nc.sync.dma_start(nsin_sb[:], nsin2[:])
            tri_sb = cpool.tile([128, 128], BF16)
            nc.sync.dma_start(tri_sb[:], trimask[:])
            ones_sb = cpool.tile([1, 64], F32)
            nc.vector.memset(ones_sb[:], 1.0)

            hid_sb = hpool.tile([128, NKT, S], BF16)
            nc.sync.dma_start(hid_sb[:], hidT.rearrange("(k p) s -> p k s", p=128))

            # persistent activations
            qT = [cpool.tile([128, S], BF16, tag=f"qT{m}", name=f"qT{m}")
                  for m in range(4)]
            kT = cpool.tile([128, S], BF16, tag="kT")
            v1 = cpool.tile([128, NST, HD + 1], BF16, tag="v1")
            ctx = [cpool.tile([128, S], BF16, tag=f"ctx{m}", name=f"ctx{m}")
                   for m in range(4)]

            def rope(dst, ps, nb):
                """dst[:, nb*512:+512] = rope(ps) with ps=[128,512] psum."""
                sl = slice(nb * 512, nb * 512 + 512)
                t1 = wpool.tile([128, 512], F32, tag="t1")
                nc.vector.tensor_tensor(t1[:], ps[:], cos_sb[:, sl], MUL)
                t2 = wpool.tile([128, 512], F32, tag="t2")
                for g in (0, 64):
                    nc.vector.tensor_tensor(
                        t2[g : g + 32], ps[g + 32 : g + 64],
                        nsin_sb[g : g + 32, sl], MUL)
                    nc.vector.tensor_tensor(
                        t2[g + 32 : g + 64], ps[g : g + 32],
                        nsin_sb[g + 32 : g + 64, sl], MUL)
                nc.vector.tensor_add(dst[:, sl], t1[:], t2[:])

            def proj_rope(dst, w_sb, wsl):
                """dst[:, :] = rope(hid.T @ w[:, wsl]) over all seq blocks."""
                for nb in range(NSB):
                    ps = psM.tile([128, 512], F32, tag="mm", name="ps_proj")
                    for kt in range(NKT):
                        nc.tensor.matmul(
                            ps[:], w_sb[:, kt, wsl],
                            hid_sb[:, kt, nb * 512 : nb * 512 + 512],
                            start=(kt == 0), stop=(kt == NKT - 1))
                    rope(dst, ps, nb)

            def attention(hp):
                """Scores/exp/ctx for head pair hp; writes ctx[hp]."""
                for ib in range(NSB):
                    isl = slice(ib * 512, ib * 512 + 512)
                    n_jt = 4 * (ib + 1)
                    pc = [psC.tile([128, 512], F32, tag="ctx", name=f"pc{i}")
                          for i in range(2)]
                    for jt in range(n_jt):
                        jsl = slice(jt * 128, jt * 128 + 128)
                        r = jt - 4 * ib          # >=0 on diagonal 512-block
                        c0 = 128 * r if r > 0 else 0   # first valid column
                        csl = slice(c0, 512)
                        isl_c = slice(ib * 512 + c0, ib * 512 + 512)
                        for h2 in range(2):
                            g = slice(h2 * 64, h2 * 64 + 64)
                            pss = psS.tile([128, 512], F32, tag="sc",
                                           name="pss")
                            nc.tensor.matmul(
                                pss[:, csl], kT[g, jsl], qT[hp][g, isl_c],
                                start=True, stop=True,
                                tile_position=(h2 * 64, 0))
                            ex = epool.tile([128, 512], BF16, tag="ex",
                                            name="ex")
                            nc.scalar.activation(ex[:, csl], pss[:, csl],
                                                 EXP, scale=SCALE)
                            if r >= 0:
                                nc.vector.tensor_tensor(
                                    ex[:, c0 : c0 + 128], ex[:, c0 : c0 + 128],
                                    tri_sb[:], MUL)
                            nc.tensor.matmul(
                                pc[h2][: HD + 1, csl], v1[:, jt, :],
                                ex[:, csl],
                                start=(jt == 0), stop=(jt == n_jt - 1))
                    for h2 in range(2):
                        rec = wpool.tile([1, 512], F32, tag="rec")
                        nc.vector.reciprocal(rec[:], pc[h2][HD : HD + 1, :])
                        pb = psM.tile([64, 512], F32, tag="mm", name="pb")
                        nc.tensor.matmul(pb[:], ones_sb[:], rec[:],
                                         start=True, stop=True)
                        pbs = wpool.tile([64, 512], F32, tag="pbs")
                        nc.vector.tensor_copy(pbs[:], pb[:])
                        nc.vector.tensor_tensor(
                            ctx[hp][h2 * 64 : h2 * 64 + 64, isl],
                            pc[h2][:HD], pbs[:], MUL)

            # ---- emission: k/v/q0 first, then attention interleaved ------
            proj_rope(kT, wkk_sb, slice(0, 128))

            for st in range(NST):  # v natural layout + ones column
                ps = psM.tile([128, 512], F32, tag="mm", name="ps_v")
                for kt in range(NKT):
                    nc.tensor.matmul(
                        ps[:, :HD], hid_sb[:, kt, st * 128 : st * 128 + 128],
                        wv_sb[:, kt, :],
                        start=(kt == 0), stop=(kt == NKT - 1))
                nc.vector.tensor_copy(v1[:, st, :HD], ps[:, :HD])
                nc.vector.memset(v1[:, st, HD:], 1.0)

            for hp in range(4):
                proj_rope(qT[hp], wq_sb, slice(hp * 128, hp * 128 + 128))
                attention(hp)

            # ---- dense ----------------------------------------------------
            for it in range(NST):
                tsl = slice(it * 128, it * 128 + 128)
                for nb in range(NSB):
                    ps = psM.tile([128, 512], F32, tag="mm", name="ps_d")
                    for kt in range(4):
                        nc.tensor.matmul(
                            ps[:], ctx[kt][:, tsl],
                            wd_sb[:, kt, nb * 512 : nb * 512 + 512],
                            start=(kt == 0), stop=(kt == 3))
                    ob = wpool.tile([128, 512], BF16, tag="ob")
                    nc.vector.tensor_copy(ob[:], ps[:])
                    nc.sync.dma_start(out[tsl, nb * 512 : nb * 512 + 512], ob[:])

    nc.compile()
    return nc


def _get_prog():
    global _prog
    if _prog is None:
        _prog = _build_program()
    return _prog


def kernel(hidden_states, w_qkv, w_dense):
    hidden_states = np.asarray(hidden_states, dtype=np.float32)
    w_qkv = np.asarray(w_qkv, dtype=np.float32)
    w_dense = np.asarray(w_dense, dtype=np.float32)

    nc = _get_prog()
    from concourse.bass_utils import run_bass_kernel_spmd

    cos2, nsin2, tri = _host_tables()
    w_k = w_qkv[:, NH * HD : NH * HD + HD]
    w_v = w_qkv[:, NH * HD + HD :]
    w_kk = _bf16(np.concatenate([w_k, w_k], axis=1))
    w_v = _bf16(w_v)

    in_maps = []
    for core in range(N_CORES):
        b, t = divmod(core, TP)
        c0 = t * QCOLS
        in_maps.append({
            "hidT": _bf16(hidden_states[b].T),
            "w_q": _bf16(w_qkv[:, c0 : c0 + QCOLS]),
            "w_kk": w_kk,
            "w_v": w_v,
            "w_d": _bf16(w_dense[c0 : c0 + QCOLS, :]),
            "cos2": cos2,
            "nsin2": nsin2,
            "trimask": tri,
        })

    trace = os.environ.get("BASS_KERNEL_TRACE", "") == "1"
    kw = {}
    if trace:
        import tempfile

        base = os.environ.get("BASS_KERNEL_TRACE_DIR")
        if base:
            os.makedirs(base, exist_ok=True)
        kw = {"trace": True, "tmpdir": tempfile.mkdtemp(dir=base or None)}
    res = run_bass_kernel_spmd(nc, in_maps, list(range(N_CORES)), **kw)
    global last_exec_time_ns
    last_exec_time_ns = res.exec_time_ns

    outp = np.zeros((B, S, HID), dtype=np.float32)
    for core in range(N_CORES):
        b = core // TP
        outp[b] += np.asarray(res.results[core]["out"], dtype=np.float32)
    return outp
